# revision 1
# baseline (speedup 1.0000x reference)
"""Bidirectional LSTM on 8 Trainium2 NeuronCores.

Sharding: data-parallel over batch B=64 -> 8 cores x 8; LSTM weights
replicated. Both directions run on every core (bwd direction is
time-reversed on the host so the device always scans forward).

Device program per core (fp32 I/O, fp32r matmuls):
  Phase 1: xW = x @ W_ih.T + (b_ih + b_hh) for both dirs, batch-major
           GEMM -> DRAM scratch chunk tiles interleaved [t, fwd8|bwd8, 1024].
  Phase 2: 512 fully-unrolled recurrence steps. Gates PSUM [16, 1024]
           (rows 0:8 fwd, 8:16 bwd), moving operand = W_hh.T (fp32r,
           N=512 chunks), stationary = h.T [128, 8] slices. Shared DVE
           add (+xW), shared sigmoid/tanh, DVE cell update, PE transpose
           of h [16,128] -> [128,16] to rebuild h.T for the next step.

Gate order is host-permuted to [i, f, o, g] so sigmoid covers gates
[0:768] and tanh covers [768:1024] in single ACT ops.
"""

import sys

sys.path.insert(0, "/opt/trn_rl_repo")

import numpy as np

L, B, D, H = 512, 64, 512, 512
HALF = H // 2
G = 4 * HALF  # 1024
NCORES = 8
BC = B // NCORES  # 8 batch rows per core
KD = D // 128  # 4 contraction chunks for the input projection
KH = HALF // 128  # 2 contraction chunks for the recurrence
NCH = 16  # timesteps per xw DRAM chunk tile
NCHUNK = L // NCH  # 32 chunk tiles per core
OUTB = 8  # timesteps buffered per output DMA
XWB = 2  # timesteps per xw prefetch block
RB = (0, 32)  # partition row-base per direction (matmul out base must be 0/32/64)
RW = 40  # partition span of step tiles (rows 0:8 fwd, 32:40 bwd)

_BUILT = None


def _build(reps: int = 1):
    import concourse.bacc as bacc
    import concourse.mybir as mybir
    import concourse.tile as tile

    F32 = mybir.dt.float32
    F32R = mybir.dt.float32r
    AF = mybir.ActivationFunctionType

    nc = bacc.Bacc(None, target_bir_lowering=False)

    # ---- DRAM I/O ----
    xT_f = nc.dram_tensor("xT_f", [D, L * BC], F32R, kind="ExternalInput")
    xT_b = nc.dram_tensor("xT_b", [D, L * BC], F32R, kind="ExternalInput")
    wih = nc.dram_tensor("wih", [2, D, G], F32R, kind="ExternalInput")
    whh = nc.dram_tensor("whh", [2, HALF, G], F32R, kind="ExternalInput")
    bias = nc.dram_tensor("bias", [2, 128, G], F32, kind="ExternalInput")
    identr = nc.dram_tensor("identr", [BC, BC], F32R, kind="ExternalInput")
    y_f = nc.dram_tensor("y_f", [L, BC, HALF], F32, kind="ExternalOutput")
    y_b = nc.dram_tensor("y_b", [L, BC, HALF], F32, kind="ExternalOutput")
    dbg_xw = nc.dram_tensor("dbg_xw", [NCH, 2 * BC, G], F32, kind="ExternalOutput")
    dbg_gss = nc.dram_tensor("dbg_gss", [BC, G], F32, kind="ExternalOutput")
    dbg_h0 = nc.dram_tensor("dbg_h0", [BC, HALF], F32, kind="ExternalOutput")
    dbg_xt = nc.dram_tensor("dbg_xt", [128, KD, 128], F32, kind="ExternalOutput")
    dbg_wih = nc.dram_tensor("dbg_wih", [128, KD, G], F32, kind="ExternalOutput")
    dbg_ot = nc.dram_tensor("dbg_ot", [128, G], F32, kind="ExternalOutput")

    with tile.TileContext(nc) as tc:
        with (
            tc.tile_pool(name="singles", bufs=1) as singles,
            tc.tile_pool(name="dram", bufs=2 * NCHUNK + 2, space="DRAM") as dram_pool,
        ):
            # Resident weights / bias / identity
            wih_sb = singles.tile([128, 2, KD, G], F32R)
            whh_sb = singles.tile([128, 2, KH, G], F32R)
            bias_sb = singles.tile([128, 2, G], F32)
            ident = singles.tile([BC, BC], F32)
            identr_sb = singles.tile([BC, BC], F32R)
            nc.sync.dma_start(identr_sb[:], identr[:, :])
            for d in range(2):
                for k in range(KD):
                    nc.sync.dma_start(
                        wih_sb[:, d, k, :], wih[d, k * 128 : (k + 1) * 128, :]
                    )
                for k in range(KH):
                    nc.sync.dma_start(
                        whh_sb[:, d, k, :], whh[d, k * 128 : (k + 1) * 128, :]
                    )
                nc.sync.dma_start(bias_sb[:, d, :], bias[d])
            from concourse.masks import make_identity

            make_identity(nc, ident[:])

            for _rep in range(reps):
                # xw scratch chunk tiles: [NCH timesteps, 16 rows, G]
                xw_tiles = [
                    dram_pool.tile([NCH, 2 * BC, G], F32R, tag="xw", name=f"xw{c}")
                    for c in range(NCHUNK)
                ]

                with (
                    tc.tile_pool(name="p1x", bufs=2) as p1x,
                    tc.tile_pool(name="p1o", bufs=2) as p1o,
                    tc.tile_pool(name="xwstep", bufs=2) as xwp,
                    tc.tile_pool(name="gsum", bufs=3) as gsump,
                    tc.tile_pool(name="gss", bufs=3) as gssp,
                    tc.tile_pool(name="small", bufs=3) as smallp,
                    tc.tile_pool(name="hout", bufs=2) as houtp,
                    tc.tile_pool(name="hT", bufs=2) as hTp,
                    tc.tile_pool(name="cstate", bufs=1) as cp,
                    tc.tile_pool(name="p1p", bufs=1, space="PSUM") as p1p,
                    tc.tile_pool(name="p2g", bufs=2, space="PSUM") as p2g,
                    tc.tile_pool(name="p2t", bufs=1, space="PSUM") as p2t,
                ):
                    def proj_chunk(c):
                        # input projection for timestep chunk c, both dirs
                        for d, xT in ((0, xT_f), (1, xT_b)):
                            xt = p1x.tile([128, KD, 128], F32R, name="xt")
                            nc.sync.dma_start(
                                xt[:],
                                xT.rearrange("(k p) n -> p k n", p=128)[
                                    :, :, c * 128 : (c + 1) * 128
                                ],
                            )
                            ps1 = p1p.tile([128, G], F32, name="ps1")
                            for n in range(2):
                                for k in range(KD):
                                    nc.tensor.matmul(
                                        ps1[:, n * 512 : (n + 1) * 512],
                                        xt[:, k, :],
                                        wih_sb[:, d, k, n * 512 : (n + 1) * 512],
                                        start=(k == 0),
                                        stop=(k == KD - 1),
                                    )
                            ot = p1o.tile([128, G], F32R, name="ot")
                            nc.vector.tensor_add(ot[:], ps1[:], bias_sb[:, d, :])
                            nc.sync.dma_start(
                                xw_tiles[c][:, d * BC : (d + 1) * BC, :], ot[:]
                            )
                            if c == 0 and d == 0 and _rep == 0:
                                nc.sync.dma_start(dbg_xt[:, :, :], xt[:].bitcast(F32))
                                nc.sync.dma_start(dbg_wih[:, :, :], wih_sb[:, 0, :, :].bitcast(F32))
                                nc.sync.dma_start(dbg_ot[:, :], ot[:].bitcast(F32))

                    PROJ_AHEAD = 2
                    for c in range(PROJ_AHEAD):
                        proj_chunk(c)

                    c_t = [cp.tile([BC, HALF], F32, tag=f"c{d}", name=f"c{d}") for d in range(2)]
                    hT = [None, None]
                    hout = [None, None]
                    xwblk = [None, None]
                    for i in range(L):
                        if i % NCH == 0 and i // NCH + PROJ_AHEAD < NCHUNK:
                            proj_chunk(i // NCH + PROJ_AHEAD)
                        for d in range(2):
                            if i % XWB == 0:
                                xwblk[d] = xwp.tile([BC, XWB, G], F32R, tag=f"xw{d}", name=f"xwb{d}")
                                ch, t0 = i // NCH, (i % NCH)
                                nc.sync.dma_start(
                                    xwblk[d][:],
                                    xw_tiles[ch][
                                        t0 : t0 + XWB, d * BC : (d + 1) * BC, :
                                    ].rearrange("t b g -> b t g"),
                                )
                            if i % OUTB == 0:
                                hout[d] = houtp.tile([BC, OUTB, HALF], F32, tag=f"ho{d}", name=f"ho{d}")
                            xw = xwblk[d][:, i % XWB, :]
                            ps = p2g.tile([BC, G], F32, tag=f"ps{d}", name=f"ps{d}", bufs=1)
                            if i > 0:
                                for n in range(2):
                                    for k in range(KH):
                                        nc.tensor.matmul(
                                            ps[:, n * 512 : (n + 1) * 512],
                                            hT[d][:, k, :],
                                            whh_sb[:, d, k, n * 512 : (n + 1) * 512],
                                            start=(k == 0),
                                            stop=False,
                                        )
                            for n in range(2):
                                nc.tensor.matmul(
                                    ps[:, n * 512 : (n + 1) * 512],
                                    identr_sb[:],
                                    xw[:, n * 512 : (n + 1) * 512],
                                    start=(i == 0),
                                    stop=True,
                                )

                            gss = gssp.tile([BC, G], F32, tag=f"gss{d}", name=f"gss{d}")
                            nc.scalar.activation(gss[:, : 3 * HALF], ps[:, : 3 * HALF], AF.Sigmoid)
                            nc.scalar.activation(gss[:, 3 * HALF :], ps[:, 3 * HALF :], AF.Tanh)

                            ig = smallp.tile([BC, HALF], F32, tag=f"ig{d}", name=f"ig{d}")
                            nc.vector.tensor_mul(ig[:], gss[:, :HALF], gss[:, 3 * HALF :])
                            if i == 0:
                                nc.vector.tensor_copy(c_t[d][:], ig[:])
                            else:
                                nc.vector.tensor_mul(c_t[d][:], gss[:, HALF : 2 * HALF], c_t[d][:])
                                nc.vector.tensor_add(c_t[d][:], c_t[d][:], ig[:])
                            tc_t = smallp.tile([BC, HALF], F32, tag=f"tc{d}", name=f"tc{d}")
                            nc.scalar.activation(tc_t[:], c_t[d][:], AF.Tanh)

                            nc.vector.tensor_mul(
                                hout[d][:, i % OUTB, :], gss[:, 2 * HALF : 3 * HALF], tc_t[:]
                            )

                            if i == 0 and d == 0 and _rep == 0:
                                nc.sync.dma_start(dbg_xw[:, :, :], xw_tiles[0][:, :, :].bitcast(F32))
                                nc.sync.dma_start(dbg_gss[:, :], gss[:])
                                nc.sync.dma_start(dbg_h0[:, :], hout[0][:, 0, :])
                            if i < L - 1:
                                pt = p2t.tile([128, KH, BC], F32, tag=f"pt{d}", name=f"pt{d}")
                                for k in range(KH):
                                    nc.tensor.transpose(
                                        pt[:, k, :],
                                        hout[d][:, i % OUTB, k * 128 : (k + 1) * 128],
                                        ident[:],
                                    )
                                hT[d] = hTp.tile([128, KH, BC], F32R, tag=f"hT{d}", name=f"hT{d}")
                                nc.vector.tensor_copy(hT[d][:], pt[:])

                        if i % OUTB == OUTB - 1:
                            t0 = i - (OUTB - 1)
                            for d, y in ((0, y_f), (1, y_b)):
                                nc.sync.dma_start(
                                    y[:, :].rearrange("t b h -> b t h")[
                                        :, t0 : t0 + OUTB, :
                                    ],
                                    hout[d][:],
                                )

    nc.finalize()
    return nc


def _get_built():
    global _BUILT
    if _BUILT is None:
        _BUILT = _build()
    return _BUILT


def kernel(x, mask, W_ih_f, W_hh_f, b_ih_f, b_hh_f, W_ih_b, W_hh_b, b_ih_b, b_hh_b):
    from concourse.bass_utils import run_bass_kernel_spmd

    x = np.asarray(x, np.float32)
    # gate reorder [i, f, g, o] -> [i, f, o, g]
    perm = np.r_[0:HALF, HALF : 2 * HALF, 3 * HALF : 4 * HALF, 2 * HALF : 3 * HALF]

    def prep(W_ih, W_hh, b_ih, b_hh):
        return (
            np.ascontiguousarray(np.asarray(W_ih, np.float32)[perm].T),
            np.ascontiguousarray(np.asarray(W_hh, np.float32)[perm].T),
            (np.asarray(b_ih, np.float32) + np.asarray(b_hh, np.float32))[perm],
        )

    wihT_f, whhT_f, bias_f = prep(W_ih_f, W_hh_f, b_ih_f, b_hh_f)
    wihT_b, whhT_b, bias_b = prep(W_ih_b, W_hh_b, b_ih_b, b_hh_b)
    wih_in = np.stack([wihT_f, wihT_b])  # [2, D, G]
    whh_in = np.stack([whhT_f, whhT_b])  # [2, HALF, G]
    bias_in = np.stack(
        [np.tile(bias_f[None, :], (128, 1)), np.tile(bias_b[None, :], (128, 1))]
    )

    # x.T per core: [D, L*BC]; bwd gets time-reversed x
    xT = np.ascontiguousarray(x.transpose(2, 0, 1))  # [D, L, B]
    xTr = np.ascontiguousarray(x[::-1].transpose(2, 0, 1))

    in_maps = []
    for c in range(NCORES):
        sl = slice(c * BC, (c + 1) * BC)
        in_maps.append(
            {
                "xT_f": np.ascontiguousarray(xT[:, :, sl]).reshape(D, L * BC),
                "xT_b": np.ascontiguousarray(xTr[:, :, sl]).reshape(D, L * BC),
                "wih": wih_in,
                "whh": whh_in,
                "bias": bias_in,
                "identr": np.eye(BC, dtype=np.float32),
            }
        )

    nc = _get_built()
    res = run_bass_kernel_spmd(nc, in_maps, core_ids=list(range(NCORES)))

    out = np.empty((L, B, H), np.float32)
    for c in range(NCORES):
        sl = slice(c * BC, (c + 1) * BC)
        out[:, sl, :HALF] = res.results[c]["y_f"]
        out[:, sl, HALF:] = res.results[c]["y_b"][::-1]
    return out



# revision 19
# speedup vs baseline: 3779.5764x; 3779.5764x over previous
"""Bidirectional LSTM on 8 Trainium2 NeuronCores.

Sharding: data-parallel over batch B=64 -> 8 cores x 8; LSTM weights
replicated. Both directions run on every core (bwd direction is
time-reversed on the host so the device always scans forward).

Device program per core (fp32r matmuls, bf16 post-activation datapath):
  Phase 1: xW = x @ W_ih.T + (b_ih + b_hh) for both dirs, batch-major
           GEMM -> DRAM scratch chunk tiles [t, fwd8|bwd8, 1024].
  Phase 2: 512 fully-unrolled recurrence steps with BOTH directions
           merged into shared ops: gates in two PSUM bank tiles
           [40, 512] each (rows 0:8 fwd, 32:40 bwd; rows 8:32 dead).
           Host permutes gates to [f, i, g, o] so bank0 = {f, i} whose
           sigmoid + f*c mul hide under bank1's matmuls; the tail after
           the last matmul is tanh(g) -> i*g -> c add -> tanh(c) ->
           h = o*tanh(c) -> PE transpose -> per-chunk hT copy.
           xw enters PSUM via one constant [16,40] scatter-permutation
           stationary (start=True, zeroes dead rows), emitted one step
           ahead so it runs during the previous step's tail.
"""

import sys

sys.path.insert(0, "/opt/trn_rl_repo")

import numpy as np

L, B, D, H = 512, 64, 512, 512
HALF = H // 2
G = 4 * HALF  # 1024
NCORES = 8
BC = B // NCORES  # 8 batch rows per core
KD = D // 128  # 4 contraction chunks for the input projection
KH = HALF // 128  # 2 contraction chunks for the recurrence
NCH = 16  # timesteps per xw DRAM chunk tile
NCHUNK = L // NCH  # 32 chunk tiles per core
OUTB = 8  # timesteps buffered per output DMA
XWB = 4  # timesteps per xw prefetch block
ROWS = 40  # partition span of step tiles (rows 0:8 fwd, 32:40 bwd)

_BUILT = {}


def _build(reps: int = 1):
    import concourse.bacc as bacc
    import concourse.mybir as mybir
    import concourse.tile as tile

    F32 = mybir.dt.float32
    F32R = mybir.dt.float32r
    BF16 = mybir.dt.bfloat16
    AF = mybir.ActivationFunctionType

    nc = bacc.Bacc(None, target_bir_lowering=False)

    # ---- DRAM I/O ----
    xT_f = nc.dram_tensor("xT_f", [D, L * BC], F32R, kind="ExternalInput")
    xT_b = nc.dram_tensor("xT_b", [D, L * BC], F32R, kind="ExternalInput")
    wih = nc.dram_tensor("wih", [2, D, G], F32R, kind="ExternalInput")
    whh = nc.dram_tensor("whh", [2, HALF, G], BF16, kind="ExternalInput")
    bias = nc.dram_tensor("bias", [2, G], F32R, kind="ExternalInput")
    perm = nc.dram_tensor("perm", [2 * BC, ROWS], F32R, kind="ExternalInput")
    ones = nc.dram_tensor("ones", [1, 128], F32R, kind="ExternalInput")
    identT = nc.dram_tensor("identT", [ROWS, ROWS], BF16, kind="ExternalInput")
    y_f = nc.dram_tensor("y_f", [L, BC, HALF], BF16, kind="ExternalOutput")
    y_b = nc.dram_tensor("y_b", [L, BC, HALF], BF16, kind="ExternalOutput")

    with tile.TileContext(nc) as tc:
        with (
            tc.tile_pool(name="singles", bufs=1) as singles,
            tc.tile_pool(name="dram", bufs=2 * NCHUNK + 2, space="DRAM") as dram_pool,
        ):
            # Resident weights / bias / permutation / transpose identity
            wih_sb = singles.tile([128, 2, KD, G], F32R)
            whh_sb = singles.tile([128, 2, KH, G], BF16)
            bias_sb = singles.tile([1, 2, G], F32R)
            ones_sb = singles.tile([1, 128], F32R)
            perm_sb = singles.tile([2 * BC, ROWS], F32R)
            identT_sb = singles.tile([ROWS, ROWS], BF16)
            nc.sync.dma_start(perm_sb[:], perm[:, :])
            nc.sync.dma_start(identT_sb[:], identT[:, :])
            nc.sync.dma_start(ones_sb[:], ones[:, :])
            for d in range(2):
                for k in range(KD):
                    nc.sync.dma_start(
                        wih_sb[:, d, k, :], wih[d, k * 128 : (k + 1) * 128, :]
                    )
                for k in range(KH):
                    nc.sync.dma_start(
                        whh_sb[:, d, k, :], whh[d, k * 128 : (k + 1) * 128, :]
                    )
                nc.sync.dma_start(bias_sb[:, d, :], bias[d : d + 1, :])

            for _rep in range(reps):
                # xw scratch chunk tiles: [NCH timesteps, 16 rows, G]
                xw_tiles = [
                    dram_pool.tile([NCH, 2 * BC, G], F32R, tag="xw", name=f"xw{c}")
                    for c in range(NCHUNK)
                ]

                with (
                    tc.tile_pool(name="p1x", bufs=2) as p1x,
                    tc.tile_pool(name="p1o", bufs=2) as p1o,
                    tc.tile_pool(name="xwstep", bufs=2) as xwp,
                    tc.tile_pool(name="gss", bufs=2) as gssp,
                    tc.tile_pool(name="small", bufs=3) as smallp,
                    tc.tile_pool(name="hout", bufs=2) as houtp,
                    tc.tile_pool(name="hT", bufs=2) as hTp,
                    tc.tile_pool(name="cstate", bufs=1) as cp,
                    tc.tile_pool(name="p1p", bufs=1, space="PSUM") as p1p,
                    tc.tile_pool(name="p2g", bufs=2, space="PSUM") as p2g,
                    tc.tile_pool(name="p2t", bufs=2, space="PSUM") as p2t,
                ):
                    def proj_chunk(c):
                        # input projection for timestep chunk c, both dirs;
                        # bias enters as a K=1 ones-row matmul, output DMAs
                        # straight from PSUM (no DVE, no SBUF staging)
                        for d, xT in ((0, xT_f), (1, xT_b)):
                            xt = p1x.tile([128, KD, 128], F32R, name="xt")
                            nc.sync.dma_start(
                                xt[:],
                                xT.rearrange("(k p) n -> p k n", p=128)[
                                    :, :, c * 128 : (c + 1) * 128
                                ],
                            )
                            ps1 = p1p.tile([128, G], F32, name="ps1")
                            for n in range(2):
                                nc.tensor.matmul(
                                    ps1[:, n * 512 : (n + 1) * 512],
                                    ones_sb[:],
                                    bias_sb[:, d, n * 512 : (n + 1) * 512],
                                    start=True,
                                    stop=False,
                                )
                                for k in range(KD):
                                    nc.tensor.matmul(
                                        ps1[:, n * 512 : (n + 1) * 512],
                                        xt[:, k, :],
                                        wih_sb[:, d, k, n * 512 : (n + 1) * 512],
                                        start=False,
                                        stop=(k == KD - 1),
                                    )
                            ot = p1o.tile([128, G], F32, name="ot")
                            nc.scalar.activation(ot[:], ps1[:], AF.Copy)
                            nc.sync.dma_start(
                                xw_tiles[c][:, d * BC : (d + 1) * BC, :],
                                ot[:].bitcast(F32R),
                            )

                    PROJ_AHEAD = 2
                    for c in range(PROJ_AHEAD):
                        proj_chunk(c)

                    def load_xwblk(j):
                        # prefetch xw block j (steps j*XWB .. j*XWB+XWB-1)
                        blk = xwp.tile([2 * BC, XWB, G], F32R, tag="xw", name="xwb")
                        ch, t0 = (j * XWB) // NCH, (j * XWB) % NCH
                        nc.sync.dma_start(
                            blk[:],
                            xw_tiles[ch][t0 : t0 + XWB, :, :].rearrange(
                                "t b g -> b t g"
                            ),
                        )
                        return blk

                    def perm_mms(i, blk):
                        # xw scatter for step i into a fresh psum pair
                        xw = blk[:, i % XWB, :]
                        pair = []
                        for n in range(2):
                            psn = p2g.tile(
                                [ROWS, 512], F32, tag=f"ps{n}", name=f"ps{n}"
                            )
                            nc.tensor.matmul(
                                psn[:],
                                perm_sb[:],
                                xw[:, n * 512 : (n + 1) * 512],
                                start=True,
                                stop=(i == 0),
                            )
                            pair.append(psn)
                        return pair

                    c_t = cp.tile([ROWS, HALF], F32, tag="c", name="c")
                    hT = None
                    hout = None
                    xwblk = [load_xwblk(0), None]
                    ps_cur = perm_mms(0, xwblk[0])
                    for i in range(L):
                        if i % XWB == 0 and (i + XWB) < L:
                            xwblk[(i // XWB + 1) % 2] = load_xwblk(i // XWB + 1)
                        if i % OUTB == 0:
                            hout = houtp.tile(
                                [ROWS, OUTB, HALF], BF16, tag="ho", name="ho"
                            )
                        ps0, ps1g = ps_cur
                        if i > 0:
                            # hh matmuls accumulate on top of the xw scatter,
                            # k-major so k=0 starts as soon as hT's k0 half
                            # from the previous step is copied
                            for k in range(KH):
                                for n, psn in ((0, ps0), (1, ps1g)):
                                    for d in range(2):
                                        nc.tensor.matmul(
                                            psn[d * 32 : d * 32 + BC, :],
                                            hT[k][:, d * 32 : d * 32 + BC],
                                            whh_sb[
                                                :, d, k, n * 512 : (n + 1) * 512
                                            ],
                                            start=False,
                                            stop=(k == KH - 1),
                                        )

                        gss = gssp.tile([ROWS, G], BF16, tag="gss", name="gss")
                        # bank0 = {f, i}: one sigmoid, hidden under bank1 mms
                        nc.scalar.activation(
                            gss[:, : 2 * HALF], ps0[:], AF.Sigmoid
                        )
                        # bank1 = {g, o}: tanh(g) on the tail; sig(o) fills the
                        # ACT gap between tanh(g) and tanh(c)
                        nc.scalar.activation(
                            gss[:, 2 * HALF : 3 * HALF], ps1g[:, :HALF], AF.Tanh
                        )
                        nc.scalar.activation(
                            gss[:, 3 * HALF :], ps1g[:, HALF:], AF.Sigmoid
                        )

                        # prefetch next step's xw scatter while this tail runs
                        if i + 1 < L:
                            ps_cur = perm_mms(i + 1, xwblk[((i + 1) // XWB) % 2])
                        # projection matmuls for a future chunk run in the
                        # PE-idle window of this step's tail
                        if i % NCH == 0 and i // NCH + PROJ_AHEAD < NCHUNK:
                            proj_chunk(i // NCH + PROJ_AHEAD)

                        ig = smallp.tile([ROWS, HALF], BF16, tag="ig", name="ig")
                        tc_t = smallp.tile([ROWS, HALF], BF16, tag="tc", name="tc")
                        if i == 0:
                            nc.vector.tensor_mul(
                                ig[:], gss[:, HALF : 2 * HALF], gss[:, 2 * HALF : 3 * HALF]
                            )
                            nc.vector.tensor_copy(c_t[:], ig[:])
                        else:
                            # f*c over the full row: hidden (f ready early)
                            nc.vector.tensor_mul(c_t[:], gss[:, :HALF], c_t[:])
                            nc.vector.tensor_mul(
                                ig[:], gss[:, HALF : 2 * HALF], gss[:, 2 * HALF : 3 * HALF]
                            )
                            nc.vector.tensor_add(c_t[:], c_t[:], ig[:])
                        nc.scalar.activation(tc_t[:], c_t[:], AF.Tanh)
                        nc.vector.tensor_mul(
                            hout[:, i % OUTB, :], gss[:, 3 * HALF :], tc_t[:]
                        )
                        if i < L - 1:
                            # transpose/copy per 128-column chunk so the k0
                            # feedback lands early and k0 matmuls start sooner
                            pt = p2t.tile([128, KH, ROWS], BF16, tag="pt", name="pt")
                            hT = [
                                hTp.tile([128, ROWS], BF16, tag=f"hT{k}", name=f"hT{k}")
                                for k in range(KH)
                            ]
                            for k in range(KH):
                                nc.tensor.transpose(
                                    pt[:, k, :],
                                    hout[:, i % OUTB, k * 128 : (k + 1) * 128],
                                    identT_sb[:],
                                )
                                nc.vector.tensor_copy(hT[k][:], pt[:, k, :])

                        if i % OUTB == OUTB - 1:
                            t0 = i - (OUTB - 1)
                            nc.sync.dma_start(
                                y_f[:, :].rearrange("t b h -> b t h")[
                                    :, t0 : t0 + OUTB, :
                                ],
                                hout[:BC, :, :],
                            )
                            nc.sync.dma_start(
                                y_b[:, :].rearrange("t b h -> b t h")[
                                    :, t0 : t0 + OUTB, :
                                ],
                                hout[32:, :, :],
                            )

    nc.finalize()
    return nc


def _get_built(reps: int = 1):
    if reps not in _BUILT:
        _BUILT[reps] = _build(reps)
    return _BUILT[reps]


def _prep_inputs(x, W_ih_f, W_hh_f, b_ih_f, b_hh_f, W_ih_b, W_hh_b, b_ih_b, b_hh_b):
    import ml_dtypes

    x = np.asarray(x, np.float32)
    # gate reorder [i, f, g, o] -> [f, i, g, o]
    gp = np.r_[HALF : 2 * HALF, 0:HALF, 2 * HALF : 3 * HALF, 3 * HALF : 4 * HALF]

    def prep(W_ih, W_hh, b_ih, b_hh):
        return (
            np.ascontiguousarray(np.asarray(W_ih, np.float32)[gp].T),
            np.ascontiguousarray(np.asarray(W_hh, np.float32)[gp].T).astype(ml_dtypes.bfloat16),
            (np.asarray(b_ih, np.float32) + np.asarray(b_hh, np.float32))[gp],
        )

    wihT_f, whhT_f, bias_f = prep(W_ih_f, W_hh_f, b_ih_f, b_hh_f)
    wihT_b, whhT_b, bias_b = prep(W_ih_b, W_hh_b, b_ih_b, b_hh_b)
    wih_in = np.stack([wihT_f, wihT_b])  # [2, D, G]
    whh_in = np.stack([whhT_f, whhT_b])  # [2, HALF, G]
    bias_in = np.stack([bias_f, bias_b])  # [2, G]

    # scatter rows 0:8 -> 0:8 (fwd) and 8:16 -> 32:40 (bwd)
    perm_in = np.zeros((2 * BC, ROWS), np.float32)
    perm_in[0:BC, 0:BC] = np.eye(BC)
    perm_in[BC : 2 * BC, 32:ROWS] = np.eye(BC)
    identT_in = np.eye(ROWS, dtype=ml_dtypes.bfloat16)

    # x.T per core: [D, L*BC]; bwd gets time-reversed x
    xT = np.ascontiguousarray(x.transpose(2, 0, 1))  # [D, L, B]
    xTr = np.ascontiguousarray(x[::-1].transpose(2, 0, 1))

    in_maps = []
    for c in range(NCORES):
        sl = slice(c * BC, (c + 1) * BC)
        in_maps.append(
            {
                "xT_f": np.ascontiguousarray(xT[:, :, sl]).reshape(D, L * BC),
                "xT_b": np.ascontiguousarray(xTr[:, :, sl]).reshape(D, L * BC),
                "wih": wih_in,
                "whh": whh_in,
                "bias": bias_in,
                "perm": perm_in,
                "ones": np.ones((1, 128), np.float32),
                "identT": identT_in,
            }
        )
    return in_maps


def _assemble(results):
    out = np.empty((L, B, H), np.float32)
    for c in range(NCORES):
        sl = slice(c * BC, (c + 1) * BC)
        out[:, sl, :HALF] = np.asarray(results[c]["y_f"]).astype(np.float32)
        out[:, sl, HALF:] = np.asarray(results[c]["y_b"]).astype(np.float32)[::-1]
    return out


def kernel(x, mask, W_ih_f, W_hh_f, b_ih_f, b_hh_f, W_ih_b, W_hh_b, b_ih_b, b_hh_b):
    from concourse.bass_utils import run_bass_kernel_spmd

    in_maps = _prep_inputs(
        x, W_ih_f, W_hh_f, b_ih_f, b_hh_f, W_ih_b, W_hh_b, b_ih_b, b_hh_b
    )
    nc = _get_built()
    res = run_bass_kernel_spmd(nc, in_maps, core_ids=list(range(NCORES)))
    return _assemble(res.results)
